# revision 30
# baseline (speedup 1.0000x reference)
"""Trainium2 Bass kernel for DiffuserAttention (GNN message passing).

v4: dual-stream selection.  The SWDGE dma_gather descriptor generation on
GPSIMD (~9ns/edge-row, serial) is the kernel bottleneck, so per-edge
h[src] row selection is split across two engines:

- gather blocks (dst blocks 0..NGATH-1): SWDGE dma_gather from the DRAM
  h table, as before;
- select blocks (dst blocks NGATH..7): one-hot *select* matmuls on the
  Tensor engine against an SBUF-resident fp8 copy of the h table.  Edges
  are placed into chunks whose sorted src values fit a static 256-node
  window, so one fp8 DoubleRow matmul (2 k-tiles) per 512-psum-column
  selects the chunk's rows.  The window one-hots (ohsel), scatter
  one-hots (oh) and their transposes (ohT, for q[dst] selection at step
  0) are host-precomputed fp8 matrices streamed from DRAM.

Step 0 gathers [k|v] rows for ALL blocks (GPSIMD has headroom there) and
computes edge scores: q[dst] via ohT matmul from the local q block, DVE
product + two folds, GPSIMD tail reduce, exp on the Scalar engine.
Edge softmax is unnormalized (escale = 7.2*exp(score)); 0.9/denom is
applied per-dst after each segment-sum.  Per-block partial AllGathers
(Shared scratchpad) hide the collective latency under the step tail.
"""

import sys

sys.path.insert(0, "/opt/trn_rl_repo")

import math

import numpy as np
import ml_dtypes

import concourse.bass as bass
import concourse.bacc as bacc
import concourse.mybir as mybir
import concourse.tile as tile
from concourse.bass_utils import run_bass_kernel_spmd

F32 = mybir.dt.float32
BF16 = mybir.dt.bfloat16
FP8 = mybir.dt.float8e4
I16 = mybir.dt.int16
NPBF16 = ml_dtypes.bfloat16
NPFP8 = ml_dtypes.float8_e4m3

NCORES = 8
NRANGE = 8
ALPHA = 0.1
NSTEPS = 5
LN_EPS = 1e-12
GCH = 8      # chunks per gather group (1024 idx = SWDGE ring capacity)
KAPPA = 8.0  # escale prescale (cancelled by rdenom); keeps msg in range
NGATH = 5    # dst blocks served by SWDGE gathers in steps 1-4
NSEL = NRANGE - NGATH


def _cfg(B, S, D, H, E):
    N = B * S
    cfg = dict(
        B=B, S=S, D=D, H=H, E=E, N=N,
        HD=D // H,
        NR=N // NRANGE,
    )
    cfg["NBLK"] = cfg["NR"] // 128
    cfg["DC"] = D // 128
    return cfg


def wrap_idx(idx):
    """dma_gather index layout: [128, n/16] int16; idx i at [i%16, i//16],
    replicated across the 8 Q7 cores."""
    n = idx.shape[0]
    w = idx.reshape(n // 16, 16).T.astype(np.int16)
    return np.ascontiguousarray(np.tile(w, (8, 1)))


def window_starts(C_BLK_S, n_tiles=64):
    return [min(int(round(j * (n_tiles - 2) / (C_BLK_S - 1))), n_tiles - 2)
            for j in range(C_BLK_S)]


def place_select(srcs, C_BLK_S, t0s):
    """Assign sorted-src edge positions to C_BLK_S chunks of <=128 edges,
    chunk j constrained to src in [t0s[j]*128, t0s[j]*128+256).  Returns
    per-chunk lists of indices into srcs, or None on failure."""
    out = [[] for _ in range(C_BLK_S)]
    i, n = 0, len(srcs)
    for j in range(C_BLK_S):
        lo, hi = t0s[j] * 128, (t0s[j] + 2) * 128
        while i < n and len(out[j]) < 128 and srcs[i] < hi:
            if srcs[i] < lo:
                return None
            out[j].append(i)
            i += 1
    if i < n:
        return None
    return out


def host_prep(cfg, hidden_states, attention_mask, src, dst,
              Wq, bq, Wk, bk, Wv, bv, Wo, bo, ln_g, ln_b):
    N, D, H, HD = cfg["N"], cfg["D"], cfg["H"], cfg["HD"]
    NR, NBLK = cfg["NR"], cfg["NBLK"]

    x = np.asarray(hidden_states, np.float32).reshape(N, D)
    src = np.asarray(src).astype(np.int64)
    dst = np.asarray(dst).astype(np.int64)
    mask1 = np.asarray(attention_mask).reshape(-1) >= 0
    all_valid = bool(mask1.all())

    # ---- edge partition by dst range, then by 128-dst block; within a
    # block sort by src (gather locality / select window placement)
    # h/kv tables are stored block-major (blk*1024 + rank*128 + row) so
    # per-block partial AllGathers write contiguous slices; all gather
    # indices and select windows live in permuted table space
    psrc = ((src % NR) // 128) * NR + (src // NR) * 128 + (src % 128)
    per_range = []
    maxchunks = 0
    for r in range(NRANGE):
        sel = np.nonzero((dst >= r * NR) & (dst < (r + 1) * NR))[0]
        dl = dst[sel] - r * NR
        order = np.lexsort((psrc[sel], dl >> 7))
        sel = sel[order]
        dl = dl[order]
        counts = np.bincount(dl >> 7, minlength=NBLK)
        maxchunks = max(maxchunks, int(np.ceil(counts / 128).max()))
        per_range.append((sel, dl, counts))

    C_BLK = maxchunks
    # select-block chunk count: multiple of 8, with placement slack
    C_BLK_S = ((C_BLK + 4 + 7) // 8) * 8
    t0s = window_starts(C_BLK_S)

    # chunk-id layout: [0, NGATH*C_BLK) gather chunks, pad to group
    # boundary with dummies, then NSEL*C_BLK_S select chunks
    while True:
        NGG = (NGATH * C_BLK + GCH - 1) // GCH
        NGG8 = NGG * GCH
        NCHUNK = NGG8 + NSEL * C_BLK_S
        EP = NCHUNK * 128

        edges = []
        ok = True
        for r in range(NRANGE):
            sel, dl, counts = per_range[r]
            srco_e = np.zeros(EP, np.int16)
            src_e = np.zeros(EP, np.int16)
            dstloc_e = np.zeros(EP, np.int64)
            live_e = np.zeros(EP, bool)
            valid_e = np.zeros(EP, bool)
            ohsel = np.zeros((NSEL * C_BLK_S, 2, 128, 128), NPFP8)
            starts = np.concatenate([[0], np.cumsum(counts)])
            for b in range(NBLK):
                s0, s1 = starts[b], starts[b + 1]
                bsel = sel[s0:s1]
                bdl = dl[s0:s1] - b * 128
                bsrc = psrc[bsel]
                bsrco = src[bsel]
                bval = (np.ones(len(bsel), bool) if all_valid
                        else (mask1[src[bsel]] & mask1[dst[bsel]]))
                if b < NGATH:
                    for j in range(C_BLK):
                        cid = b * C_BLK + j
                        o = cid * 128
                        pos = slice(j * 128, min((j + 1) * 128, len(bsel)))
                        n = max(0, pos.stop - pos.start)
                        if n <= 0:
                            continue
                        src_e[o:o + n] = bsrc[pos]
                        srco_e[o:o + n] = bsrco[pos]
                        dstloc_e[o:o + n] = bdl[pos]
                        live_e[o:o + n] = True
                        valid_e[o:o + n] = bval[pos]
                else:
                    placed = place_select(bsrc, C_BLK_S, t0s)
                    if placed is None:
                        ok = False
                        break
                    sb = b - NGATH
                    for j in range(C_BLK_S):
                        cid = NGG8 + (sb * C_BLK_S + j)
                        o = cid * 128
                        idxs = placed[j]
                        n = len(idxs)
                        if n == 0:
                            continue
                        src_e[o:o + n] = bsrc[idxs]
                        srco_e[o:o + n] = bsrco[idxs]
                        dstloc_e[o:o + n] = bdl[idxs]
                        live_e[o:o + n] = True
                        valid_e[o:o + n] = bval[idxs]
                        scid = sb * C_BLK_S + j
                        srel = bsrc[idxs] - t0s[j] * 128
                        epos = np.arange(n)
                        ohsel[scid, srel >> 7, srel & 127, epos] = 1.0
                if not ok:
                    break
            if not ok:
                break

            oh = np.zeros(EP * 128, NPFP8)
            ee = np.nonzero(live_e & valid_e)[0]
            oh[ee * 128 + dstloc_e[ee]] = 1.0
            oh = oh.reshape(NCHUNK, 128, 128)
            ohT = np.zeros((NCHUNK, 128, 128), NPFP8)
            el = np.nonzero(live_e)[0]
            ohT[el >> 7, dstloc_e[el], el & 127] = 1.0
            edges.append(dict(
                src16=wrap_idx(srco_e),
                src16h=wrap_idx(src_e),
                oh=np.ascontiguousarray(oh.transpose(1, 0, 2)),
                ohT=np.ascontiguousarray(ohT.transpose(1, 0, 2)),
                ohsel=np.ascontiguousarray(ohsel.transpose(2, 0, 1, 3)),
            ))
        if ok:
            break
        C_BLK_S += 8
        t0s = window_starts(C_BLK_S)

    # ---- weights / constants
    scale_q = 1.0 / math.sqrt(HD)
    Wq_s = np.ascontiguousarray((np.asarray(Wq) * scale_q).astype(NPBF16))
    Wk_s = np.ascontiguousarray(np.asarray(Wk).astype(NPBF16))
    Wv_s = np.ascontiguousarray(np.asarray(Wv).astype(NPBF16))
    bias3 = np.zeros((128, 3, D), np.float32)
    bias3[:, 0, :] = np.asarray(bq) * scale_q
    bias3[:, 1, :] = bk
    bias3[:, 2, :] = bv
    Wo_bf = np.ascontiguousarray(np.asarray(Wo).astype(NPBF16))

    ident = np.eye(128, dtype=np.float32).astype(NPBF16)
    g_rep = np.ascontiguousarray(
        np.broadcast_to(np.asarray(ln_g, np.float32), (128, D)))
    b_rep = np.ascontiguousarray(
        np.broadcast_to(np.asarray(ln_b, np.float32), (128, D)))

    in_maps = []
    for c in range(NCORES):
        rows = slice(c * NR, (c + 1) * NR)
        xTown = np.ascontiguousarray(x[rows].T.astype(NPBF16))
        xb = np.ascontiguousarray(x[rows] + np.asarray(bo, np.float32))
        m = dict(
            xTown=xTown, Wq=Wq_s, Wk=Wk_s, Wv=Wv_s,
            bias3=bias3, Wo=Wo_bf, xb=xb, g_rep=g_rep, b_rep=b_rep,
            ident=ident,
            **edges[c],
        )
        in_maps.append(m)
    zero_bias = not (np.any(bias3) or False)
    meta = dict(C_BLK=C_BLK, C_BLK_S=C_BLK_S, zero_bias=zero_bias)
    return in_maps, meta


def build_program(cfg, C_BLK, C_BLK_S, zero_bias=False):
    N, D, H, HD = cfg["N"], cfg["D"], cfg["H"], cfg["HD"]
    NR, NBLK, DC = cfg["NR"], cfg["NBLK"], cfg["DC"]
    NGG = (NGATH * C_BLK + GCH - 1) // GCH
    NGG8 = NGG * GCH
    NSGC = NSEL * C_BLK_S
    NSG = NSGC // GCH
    NCHUNK = NGG8 + NSGC
    EP = NCHUNK * 128
    NTO = NR // 128
    t0s = window_starts(C_BLK_S)

    def cid_info(cid):
        """(blk, ch, cb) or None for dummy pad chunks."""
        if cid < NGATH * C_BLK:
            b, ch = divmod(cid, C_BLK)
            return b, ch, C_BLK
        if cid < NGG8:
            return None
        t = cid - NGG8
        b2, ch = divmod(t, C_BLK_S)
        return NGATH + b2, ch, C_BLK_S

    nc = bacc.Bacc(None, target_bir_lowering=False, debug=False,
                   num_devices=NCORES)

    xTown_in = nc.dram_tensor("xTown", [D, NR], BF16, kind="ExternalInput")
    Wq_in = nc.dram_tensor("Wq", [D, D], BF16, kind="ExternalInput")
    Wk_in = nc.dram_tensor("Wk", [D, D], BF16, kind="ExternalInput")
    Wv_in = nc.dram_tensor("Wv", [D, D], BF16, kind="ExternalInput")
    bias3_in = nc.dram_tensor("bias3", [128, 3, D], F32, kind="ExternalInput")
    Wo_in = nc.dram_tensor("Wo", [D, D], BF16, kind="ExternalInput")
    xb_in = nc.dram_tensor("xb", [NR, D], F32, kind="ExternalInput")
    g_rep_in = nc.dram_tensor("g_rep", [128, D], F32, kind="ExternalInput")
    b_rep_in = nc.dram_tensor("b_rep", [128, D], F32, kind="ExternalInput")
    ident_in = nc.dram_tensor("ident", [128, 128], BF16, kind="ExternalInput")
    src16_in = nc.dram_tensor("src16", [128, EP // 16], I16,
                              kind="ExternalInput")
    src16h_in = nc.dram_tensor("src16h", [128, EP // 16], I16,
                               kind="ExternalInput")
    oh_in = nc.dram_tensor("oh", [128, NCHUNK, 128], FP8,
                           kind="ExternalInput")
    ohT_in = nc.dram_tensor("ohT", [128, NCHUNK, 128], FP8,
                            kind="ExternalInput")
    ohsel_in = nc.dram_tensor("ohsel", [128, NSGC, 2, 128], FP8,
                              kind="ExternalInput")

    out_ext = nc.dram_tensor("out", [NR, D], F32, kind="ExternalOutput")

    hA = nc.dram_tensor("hAsh", [N, D], BF16, kind="Internal",
                        addr_space="Shared")
    hB = nc.dram_tensor("hBsh", [N, D], BF16, kind="Internal",
                        addr_space="Shared")
    kv_dram = nc.dram_tensor("kvsh", [N, 2 * D], BF16, kind="Internal",
                             addr_space="Shared")

    AG = [list(range(NCORES))]
    DR = mybir.MatmulPerfMode.DoubleRow

    with tile.TileContext(nc) as tc:
        with (
            tc.tile_pool(name="res", bufs=1) as res,
            tc.tile_pool(name="dram", bufs=1, space="DRAM") as dram,
        ):
            ident_sb = res.tile([128, 128], BF16)
            nc.sync.dma_start(ident_sb[:], ident_in[:])
            ln72_sb = res.tile([128, 1], F32)
            nc.vector.memset(ln72_sb[:], float(np.log(0.9 * KAPPA)))
            src16_sb = res.tile([128, EP // 16], I16)
            nc.sync.dma_start(src16_sb[:], src16_in[:])
            src16h_sb = res.tile([128, EP // 16], I16)
            nc.sync.dma_start(src16h_sb[:], src16h_in[:])
            escale_sb = res.tile([128, NCHUNK, H, 2], BF16)
            v01_sb = res.tile([128, NTO, D], BF16)
            h5_sb = res.tile([128, NTO, D], BF16)
            rdenom_sb = res.tile([128, NBLK, H], F32)

            q_dram = dram.tile([NR, D], BF16)
            kvshard = dram.tile([NR, 2 * D], BF16)
            shard = dram.tile([NR, D], BF16)

            def finalize(s, blk, psm, p2s):
                if s == 0:
                    dn = p2s.tile([128, H], F32, tag="dn")
                    nc.vector.tensor_scalar(
                        dn[:], psm[:, 768:768 + H], 1e-9, None,
                        mybir.AluOpType.max)
                    dn2 = p2s.tile([128, H], F32, tag="dn2")
                    nc.vector.reciprocal(dn2[:], dn[:])
                    nc.vector.tensor_scalar(
                        rdenom_sb[:, blk, :], dn2[:], 0.9, None,
                        mybir.AluOpType.mult)
                if s == NSTEPS - 1:
                    stg_ap = h5_sb[:, blk, :]
                else:
                    stg = p2s.tile([128, D], BF16, tag="hstg")
                    stg_ap = stg[:]
                for h in range(H):
                    hs = slice(h * HD, (h + 1) * HD)
                    nc.vector.scalar_tensor_tensor(
                        stg_ap[:, hs], psm[:, hs],
                        rdenom_sb[:, blk, h:h + 1],
                        v01_sb[:, blk, hs],
                        mybir.AluOpType.mult, mybir.AluOpType.add)
                if s < NSTEPS - 1:
                    nc.sync.dma_start(
                        shard[blk * 128:(blk + 1) * 128, :], stg_ap)
                    nc.gpsimd.collective_compute(
                        "AllGather", mybir.AluOpType.bypass,
                        replica_groups=AG,
                        ins=[shard[blk * 128:(blk + 1) * 128, :].opt()],
                        outs=[hdsts[s][
                            blk * NR:(blk + 1) * NR, :].opt()])

            def scatter(psm, oh_ap, src_ap, voff, ch, cb):
                for j in range(2):
                    js = slice(voff + j * 512, voff + min((j + 1) * 512, D))
                    os_ = slice(j * 512, min((j + 1) * 512, D))
                    nc.tensor.matmul(
                        psm[:, os_], oh_ap, src_ap[:, js],
                        start=(ch == 0), stop=(ch == cb - 1))

            # =========== P0: projections ===========
            with (
                tc.tile_pool(name="p0", bufs=3) as p0,
                tc.tile_pool(name="p0c", bufs=1) as p0c,
                tc.tile_pool(name="p0ps", bufs=4, space="PSUM") as p0ps,
            ):
                xTo_sb = p0c.tile([128, DC, NR], BF16)
                nc.sync.dma_start(
                    xTo_sb[:], xTown_in[:].rearrange("(c p) n -> p c n", p=128))
                W_sb = p0c.tile([128, 3, DC, D], BF16)
                for i, W in enumerate([Wq_in, Wk_in, Wv_in]):
                    nc.sync.dma_start(
                        W_sb[:, i, :, :],
                        W[:].rearrange("(c p) g -> p c g", p=128))
                bias_sb = p0c.tile([128, 3, D], F32)
                nc.sync.dma_start(bias_sb[:], bias3_in[:])

                def proj_tile(xsrc, toff, wi, stores, v01_t=None):
                    ps = p0ps.tile([128, D], F32, tag="pps")
                    for c in range(DC):
                        for j in range(2):
                            js = slice(j * 512, min((j + 1) * 512, D))
                            nc.tensor.matmul(
                                ps[:, js], xsrc[:, c, toff:toff + 128],
                                W_sb[:, wi, c, js],
                                start=(c == 0), stop=(c == DC - 1))
                    stg = p0.tile([128, D], BF16, tag="pstg")
                    if zero_bias:
                        nc.scalar.copy(stg[:], ps[:])
                    else:
                        nc.vector.tensor_tensor(
                            stg[:], ps[:], bias_sb[:, wi, :],
                            mybir.AluOpType.add)
                    for dest in stores:
                        nc.sync.dma_start(dest, stg[:])
                    if v01_t is not None:
                        nc.vector.tensor_scalar(
                            v01_sb[:, v01_t, :], stg[:], ALPHA, None,
                            mybir.AluOpType.mult)

                for t in range(NTO):
                    r = slice(t * 128, (t + 1) * 128)
                    proj_tile(xTo_sb, t * 128, 1, [kvshard[r, 0:D]])
                    proj_tile(xTo_sb, t * 128, 2, [kvshard[r, D:2 * D]],
                              v01_t=t)
                nc.gpsimd.collective_compute(
                    "AllGather", mybir.AluOpType.bypass,
                    replica_groups=AG,
                    ins=[kvshard[:].opt()], outs=[kv_dram[:].opt()])
                for t in range(NTO):
                    proj_tile(xTo_sb, t * 128, 0,
                              [q_dram[t * 128:(t + 1) * 128, :]])

            hsrcs = [kv_dram, hB, hA, hB, hA]
            hdsts = [hB, hA, hB, hA, None]

            # =========== P1: step 0 (scores + first diffusion step) =====
            with (
                tc.tile_pool(name="p2kv", bufs=2) as p2kv,
                tc.tile_pool(name="p2oh", bufs=3) as p2oh,
                tc.tile_pool(name="p2q", bufs=2) as p2q,
                tc.tile_pool(name="p2qb", bufs=2) as p2qb,
                tc.tile_pool(name="p2s", bufs=2) as p2s,
                tc.tile_pool(name="p2ps", bufs=2, space="PSUM") as p2ps,
                tc.tile_pool(name="p2psq", bufs=2, space="PSUM") as p2psq,
            ):
                psm = None
                qblk = None
                for g in range(NCHUNK // GCH):
                    q0 = g * GCH
                    ic = q0 * 8
                    gs = slice(q0, q0 + GCH)
                    oh = p2oh.tile([128, GCH, 128], FP8, tag="oh")
                    nc.sync.dma_start(oh[:], oh_in[:, gs, :])
                    G = p2kv.tile([128, GCH, 2 * D], BF16, tag="Gkv")
                    nc.gpsimd.dma_gather(
                        G[:], kv_dram[:], src16_sb[:, ic:ic + GCH * 8],
                        GCH * 128, GCH * 128, 2 * D)
                    ohT = p2q.tile([128, GCH, 128], FP8, tag="ohT")
                    nc.sync.dma_start(ohT[:], ohT_in[:, gs, :])
                    vG = G[:, :, D:2 * D]
                    qb16 = p2s.tile([128, GCH, D], BF16, tag="qb16")
                    for gc in range(GCH):
                        info = cid_info(q0 + gc)
                        if info is None:
                            continue
                        blk, ch, cb = info
                        if ch == 0:
                            qblk = p2qb.tile([128, D], BF16, tag="qblk")
                            nc.sync.dma_start(
                                qblk[:],
                                q_dram[blk * 128:(blk + 1) * 128, :])
                        qps = p2psq.tile([128, D], F32, tag="qps")
                        for j in range(2):
                            js = slice(j * 512, min((j + 1) * 512, D))
                            nc.tensor.matmul(
                                qps[:, js], ohT[:, gc, :], qblk[:, js],
                                start=True, stop=True)
                        nc.scalar.copy(qb16[:, gc, :], qps[:])
                    qbh = qb16[:].rearrange("p c (h f) -> p c h f", h=H)
                    nc.vector.tensor_tensor(
                        qbh,
                        G[:, :, 0:D].rearrange("p c (h f) -> p c h f", h=H),
                        qbh, mybir.AluOpType.mult)
                    for w in (32, 16, 8, 4, 2, 1):
                        nc.vector.tensor_tensor(
                            qbh[:, :, :, 0:w], qbh[:, :, :, 0:w],
                            qbh[:, :, :, w:2 * w], mybir.AluOpType.add)
                    nc.scalar.activation(
                        escale_sb[:, gs, :, :],
                        qbh[:, :, :, 0:1].to_broadcast((128, GCH, H, 2)),
                        mybir.ActivationFunctionType.Exp,
                        bias=ln72_sb[:], scale=1.0)
                    nc.vector.tensor_tensor(
                        vG.rearrange("p c (h f2 two) -> p c h f2 two",
                                     h=H, two=2),
                        vG.rearrange("p c (h f2 two) -> p c h f2 two",
                                     h=H, two=2),
                        escale_sb[:, gs, :, :].unsqueeze(3)
                        .to_broadcast((128, GCH, H, HD // 2, 2)),
                        mybir.AluOpType.mult)
                    for gc in range(GCH):
                        info = cid_info(q0 + gc)
                        if info is None:
                            continue
                        blk, ch, cb = info
                        if ch == 0:
                            psm = p2ps.tile([128, 832], F32, tag="psm")
                        scatter(psm, oh[:, gc, :], G[:, gc, :], D, ch, cb)
                        nc.tensor.matmul(
                            psm[:, 768:768 + H], oh[:, gc, :],
                            escale_sb[:, q0 + gc, :, 0:1].rearrange(
                                "p h one -> p (h one)"),
                            start=(ch == 0), stop=(ch == cb - 1),
                            skip_group_check=True)
                        if ch == cb - 1:
                            finalize(0, blk, psm, p2s)

            # =========== P2: diffusion steps 1-4, dual stream ===========
            with (
                tc.tile_pool(name="p4c", bufs=1) as p4c,
                tc.tile_pool(name="p4h", bufs=3) as p4h,
                tc.tile_pool(name="p4oh", bufs=3) as p4oh,
                tc.tile_pool(name="p4os", bufs=3) as p4os,
                tc.tile_pool(name="p4s", bufs=3) as p4s,
                tc.tile_pool(name="p4stg", bufs=2) as p4stg,
                tc.tile_pool(name="p4ps", bufs=2, space="PSUM") as p4ps,
                tc.tile_pool(name="p4psq", bufs=2, space="PSUM") as p4psq,
            ):
                h_sb = p4c.tile([128, N // 128, D], FP8)

                # interleave gather groups and select groups
                sched = []
                si = 0
                for gi in range(NGG):
                    sched.append(("g", gi))
                    while (gi + 1) * NSG // NGG > si:
                        sched.append(("s", si))
                        si += 1
                while si < NSG:
                    sched.append(("s", si))
                    si += 1

                psm = None
                for s in range(1, NSTEPS):
                    # refresh the fp8 SBUF h table from the AllGather output
                    for t8 in range(8):
                        stg8 = p4stg.tile([128, 8, D], BF16, tag="h8")
                        nc.sync.dma_start(
                            stg8[:],
                            hsrcs[s][:].rearrange(
                                "(t p) d -> p t d", p=128)[
                                :, 8 * t8:8 * (t8 + 1), :])
                        nc.scalar.copy(
                            h_sb[:, 8 * t8:8 * (t8 + 1), :], stg8[:])
                    for kind, gi in sched:
                        if kind == "g":
                            q0 = gi * GCH
                            ic = q0 * 8
                            gs = slice(q0, q0 + GCH)
                            oh = p4oh.tile([128, GCH, 128], FP8, tag="oh")
                            nc.sync.dma_start(oh[:], oh_in[:, gs, :])
                            G = p4h.tile([128, GCH, D], BF16, tag="Gh")
                            nc.gpsimd.dma_gather(
                                G[:], hsrcs[s][:],
                                src16h_sb[:, ic:ic + GCH * 8],
                                GCH * 128, GCH * 128, D)
                            nc.vector.tensor_tensor(
                                G[:].rearrange(
                                    "p c (h f2 two) -> p c h f2 two",
                                    h=H, two=2),
                                G[:].rearrange(
                                    "p c (h f2 two) -> p c h f2 two",
                                    h=H, two=2),
                                escale_sb[:, gs, :, :].unsqueeze(3)
                                .to_broadcast((128, GCH, H, HD // 2, 2)),
                                mybir.AluOpType.mult)
                            for gc in range(GCH):
                                info = cid_info(q0 + gc)
                                if info is None:
                                    continue
                                blk, ch, cb = info
                                if ch == 0:
                                    psm = p4ps.tile([128, D], F32,
                                                    tag="psm")
                                scatter(psm, oh[:, gc, :], G[:, gc, :],
                                        0, ch, cb)
                                if ch == cb - 1:
                                    finalize(s, blk, psm, p4s)
                        else:
                            sc0 = gi * GCH
                            cid0 = NGG8 + sc0
                            ohsel = p4os.tile([128, GCH, 2, 128], FP8,
                                              tag="ohsel")
                            nc.sync.dma_start(
                                ohsel[:], ohsel_in[:, sc0:sc0 + GCH, :, :])
                            oh = p4oh.tile([128, GCH, 128], FP8, tag="oh")
                            nc.sync.dma_start(
                                oh[:], oh_in[:, cid0:cid0 + GCH, :])
                            for gc in range(GCH):
                                scid = sc0 + gc
                                b2, ch = divmod(scid, C_BLK_S)
                                blk = NGATH + b2
                                t0 = t0s[ch]
                                if ch == 0:
                                    psm = p4ps.tile([128, D], F32,
                                                    tag="psm")
                                selps = p4psq.tile([128, D], F32,
                                                   tag="selps")
                                for j in range(2):
                                    js = slice(j * 512,
                                               min((j + 1) * 512, D))
                                    nc.tensor.matmul(
                                        selps[:, js],
                                        ohsel[:, gc, :, :],
                                        h_sb[:, t0:t0 + 2, js],
                                        start=True, stop=True,
                                        perf_mode=DR)
                                msg = p4s.tile([128, D], BF16, tag="msg")
                                nc.vector.tensor_tensor(
                                    msg[:].rearrange(
                                        "p (h f2 two) -> p h f2 two",
                                        h=H, two=2),
                                    selps[:].rearrange(
                                        "p (h f2 two) -> p h f2 two",
                                        h=H, two=2),
                                    escale_sb[:, cid0 + gc, :, :]
                                    .unsqueeze(2)
                                    .to_broadcast((128, H, HD // 2, 2)),
                                    mybir.AluOpType.mult)
                                scatter(psm, oh[:, gc, :], msg[:],
                                        0, ch, C_BLK_S)
                                if ch == C_BLK_S - 1:
                                    finalize(s, blk, psm, p4s)

            # =========== P3: output projection + LN ===========
            with (
                tc.tile_pool(name="p3", bufs=2) as p3,
                tc.tile_pool(name="p3c", bufs=1) as p3c,
                tc.tile_pool(name="p3ps", bufs=4, space="PSUM") as p3ps,
                tc.tile_pool(name="p3ps2", bufs=2, space="PSUM") as p3ps2,
            ):
                g_sb = p3c.tile([128, D], F32)
                nc.sync.dma_start(g_sb[:], g_rep_in[:])
                b_sb = p3c.tile([128, D], F32)
                nc.sync.dma_start(b_sb[:], b_rep_in[:])
                h5T_sb = p3c.tile([128, DC, NR], BF16)
                for t in range(NTO):
                    for c in range(DC):
                        tp = p3ps.tile([128, 128], BF16, tag="tp")
                        nc.tensor.transpose(
                            tp[:], h5_sb[:, t, c * 128:(c + 1) * 128],
                            ident_sb[:])
                        nc.vector.tensor_copy(
                            h5T_sb[:, c, t * 128:(t + 1) * 128], tp[:])
                Wo_sb = p3c.tile([128, DC, D], BF16)
                nc.sync.dma_start(
                    Wo_sb[:], Wo_in[:].rearrange("(c p) n -> p c n", p=128))
                for t in range(NTO):
                    yps = p3ps2.tile([128, D], F32, tag="yps")
                    for c in range(DC):
                        for j in range(2):
                            js = slice(j * 512, min((j + 1) * 512, D))
                            nc.tensor.matmul(
                                yps[:, js],
                                h5T_sb[:, c, t * 128:(t + 1) * 128],
                                Wo_sb[:, c, js],
                                start=(c == 0), stop=(c == DC - 1))
                    xb_sb = p3.tile([128, D], F32, tag="xb")
                    nc.sync.dma_start(xb_sb[:], xb_in[t * 128:(t + 1) * 128, :])
                    y_sb = p3.tile([128, D], F32, tag="y")
                    nc.vector.tensor_tensor(
                        y_sb[:], yps[:], xb_sb[:], mybir.AluOpType.add)
                    mu = p3.tile([128, 1], F32, tag="mu")
                    nc.vector.tensor_reduce(
                        mu[:], y_sb[:], mybir.AxisListType.X,
                        mybir.AluOpType.add)
                    negmu = p3.tile([128, 1], F32, tag="negmu")
                    nc.vector.tensor_scalar(
                        negmu[:], mu[:], -1.0 / D, None, mybir.AluOpType.mult)
                    sq = p3.tile([128, D], F32, tag="sq")
                    var = p3.tile([128, 1], F32, tag="var")
                    nc.scalar.activation(
                        sq[:], y_sb[:], mybir.ActivationFunctionType.Square,
                        bias=negmu[:], scale=1.0, accum_out=var[:])
                    vs = p3.tile([128, 1], F32, tag="vs")
                    nc.vector.tensor_scalar(
                        vs[:], var[:], 1.0 / D, LN_EPS,
                        mybir.AluOpType.mult, mybir.AluOpType.add)
                    std = p3.tile([128, 1], F32, tag="std")
                    nc.scalar.sqrt(std[:], vs[:])
                    rstd = p3.tile([128, 1], F32, tag="rstd")
                    nc.vector.reciprocal(rstd[:], std[:])
                    t1 = p3.tile([128, D], F32, tag="t1")
                    nc.vector.scalar_tensor_tensor(
                        t1[:], y_sb[:], negmu[:], g_sb[:],
                        mybir.AluOpType.add, mybir.AluOpType.mult)
                    outt = p3.tile([128, D], F32, tag="outt")
                    nc.vector.scalar_tensor_tensor(
                        outt[:], t1[:], rstd[:], b_sb[:],
                        mybir.AluOpType.mult, mybir.AluOpType.add)
                    nc.sync.dma_start(
                        out_ext[t * 128:(t + 1) * 128, :], outt[:])

    nc.compile()
    return nc


_PROG_CACHE = {}


def _get_program(cfg, C_BLK, C_BLK_S, zero_bias):
    key = (cfg["N"], cfg["E"], cfg["D"], cfg["H"], C_BLK, C_BLK_S, zero_bias)
    if key not in _PROG_CACHE:
        _PROG_CACHE[key] = build_program(cfg, C_BLK, C_BLK_S, zero_bias)
    return _PROG_CACHE[key]


def run(cfg, inputs, trace=False):
    in_maps, meta = host_prep(cfg, **inputs)
    nc = _get_program(cfg, meta["C_BLK"], meta["C_BLK_S"],
                      meta["zero_bias"])
    res = run_bass_kernel_spmd(
        nc, in_maps, core_ids=list(range(NCORES)), trace=trace)
    N, D, NR = cfg["N"], cfg["D"], cfg["NR"]
    full = np.empty((N, D), np.float32)
    for r in range(NRANGE):
        full[r * NR:(r + 1) * NR] = res.results[r]["out"]
    return full.reshape(cfg["B"], cfg["S"], D), res


def kernel(**inputs):
    cfg = _cfg(B=2, S=4096, D=768, H=12, E=524288)
    out, _ = run(cfg, inputs)
    return out
